# revision 26
# baseline (speedup 1.0000x reference)
"""CirculantLinear as a dense GEMM on 8 TRN2 NeuronCores.

Math: y[b, o] = sum_n x[b, n] * c[o, (-n) mod IN] + bias[o]
    (element 0 of the circular convolution == dot with first row of the
     circulant matrix, vectorized over outputs/batch -> one dense GEMM).

Strategy:
  - Data-parallel over batch: 8 cores x 1024 rows of x each; c/bias replicated.
  - Host-side layout prep (part of sharding): feed each core
      xT  = x_shard.T               [IN, BS]   (contraction-major)
      cT  = c[:, sigma].T           [IN, OUT]  (contraction-major, circulant
                                                column-permutation folded in)
    so the device kernel is a pure k-major GEMM with natural (non-transposed)
    DMA loads.
  - Per core: cache all of xT in SBUF (16.7 MB), stream cT once, accumulate
    out[b:128, o:512] tiles in all 8 PSUM banks, evict via DVE with the
    (partition-broadcast) bias add fused.
  - Matmuls run in float32r (full-rate fp32 tensor-engine mode, ~1e-4 rel err).
"""

import numpy as np

B, OUT, IN = 8192, 4096, 4096
NCORES = 8
BS = B // NCORES  # 1024 batch rows per core
P = 128
KT = IN // P  # 32 contraction tiles
KG = 4  # k-tiles per cT DMA (1 MiB transfers)
N_CHUNK = 512
N_CHUNKS = OUT // N_CHUNK  # 8
M_TILES = BS // P  # 8

_CACHE = {}


def _build_nc(reps=1, x_split=True, w_bufs=3, split_rings=True, kg=KG):
    """reps>1 repeats the whole compute (idempotent y writes) — used only to
    measure steady-state device time as the slope over reps. x_split loads
    xT into per-k-tile SBUF tiles so first matmuls only wait on their own
    k-slab's DMA."""
    import concourse.bacc as bacc
    import concourse.bass as bass
    import concourse.mybir as mybir
    import concourse.tile as tile

    nc = bacc.Bacc("TRN2", target_bir_lowering=False, debug=False)
    xT_d = nc.dram_tensor("xT", [IN, BS], mybir.dt.float32r, kind="ExternalInput")
    cT_d = nc.dram_tensor("cT", [IN, OUT], mybir.dt.float32r, kind="ExternalInput")
    bias_d = nc.dram_tensor("bias", [1, OUT], mybir.dt.float32, kind="ExternalInput")
    y_d = nc.dram_tensor("y", [BS, OUT], mybir.dt.float32, kind="ExternalOutput")

    with tile.TileContext(nc) as tc:
        with (
            tc.tile_pool(name="xpool", bufs=1) as xpool,
            tc.tile_pool(name="wpool", bufs=w_bufs) as wpool,
            tc.tile_pool(name="bpool", bufs=2) as bpool,
            tc.tile_pool(name="opool", bufs=4) as opool,
            tc.tile_pool(name="pspool", bufs=1, space="PSUM") as pspool,
        ):
            # two HWDGE rings: w/bias loads on SP (nc.sync), x preload and
            # output stores on ACT (nc.scalar) so they don't queue behind
            # the streaming weight loads.
            dma2 = nc.scalar if split_rings else nc.sync
            xT_r = xT_d.ap().rearrange("(ko ki) b -> ki ko b", ki=P)
            if x_split:
                # tiles allocated now; DMA issue interleaved with the first
                # n-chunk's weight loads (below) so the first matmuls don't
                # queue behind the whole 16.7MB x preload.
                xk = [
                    xpool.tile([P, BS], mybir.dt.float32r, name=f"xk_{ko}")
                    for ko in range(KT)
                ]
                xslice = lambda k, m: xk[k][:, m * P : (m + 1) * P]
            else:
                xsb = xpool.tile([P, KT, BS], mybir.dt.float32r, name="xsb")
                for ko in range(KT):
                    nc.sync.dma_start(xsb[:, ko], xT_r[:, ko])
                xslice = lambda k, m: xsb[:, k, m * P : (m + 1) * P]

            cT_r = cT_d.ap().rearrange("(ko ki) o -> ki ko o", ki=P)
            bias_ap = bias_d.ap()

            for _rep, n in [
                (r, nn) for r in range(reps) for nn in range(N_CHUNKS)
            ]:
                bias_t = bpool.tile([P, N_CHUNK], mybir.dt.float32, name="bias_t")
                bias_src = bass.AP(
                    tensor=bias_ap.tensor,
                    offset=n * N_CHUNK,
                    ap=[[0, P], [1, N_CHUNK]],
                )
                nc.sync.dma_start(bias_t, bias_src)

                psums = [
                    pspool.tile([P, N_CHUNK], mybir.dt.float32, name=f"ps_{m}")
                    for m in range(M_TILES)
                ]
                for kgi in range(KT // kg):
                    if x_split and _rep == 0 and n == 0:
                        for kk in range(kg):
                            ko = kgi * kg + kk
                            dma2.dma_start(xk[ko], xT_r[:, ko])
                    w_t = wpool.tile([P, kg, N_CHUNK], mybir.dt.float32r, name="w_t")
                    nc.sync.dma_start(
                        w_t,
                        cT_r[
                            :,
                            kgi * kg : (kgi + 1) * kg,
                            n * N_CHUNK : (n + 1) * N_CHUNK,
                        ],
                    )
                    for kk in range(kg):
                        k = kgi * kg + kk
                        for m in range(M_TILES):
                            nc.tensor.matmul(
                                psums[m],
                                xslice(k, m),
                                w_t[:, kk],
                                start=(k == 0),
                                stop=(k == KT - 1),
                            )
                for m in range(M_TILES):
                    o_t = opool.tile([P, N_CHUNK], mybir.dt.float32, name="o_t")
                    nc.vector.tensor_add(o_t, psums[m], bias_t)
                    nc.sync.dma_start(
                        y_d.ap()[
                            m * P : (m + 1) * P, n * N_CHUNK : (n + 1) * N_CHUNK
                        ],
                        o_t,
                    )
    nc.compile()
    return nc


class _Runtime:
    """Compiles the Bass program once and keeps a cached jitted SPMD callable
    (mirrors concourse.bass2jax.run_bass_via_pjrt's multi-core path)."""

    def __init__(self, reps=1, **build_kw):
        import jax
        from jax.experimental.shard_map import shard_map
        from jax.sharding import Mesh, PartitionSpec

        import concourse.mybir as mybir
        from concourse import bass2jax

        bass2jax.install_neuronx_cc_hook()
        nc = _build_nc(reps=reps, **build_kw)
        self.nc = nc

        partition_name = (
            nc.partition_id_tensor.name if nc.partition_id_tensor else None
        )
        in_names = []
        out_names = []
        out_avals = []
        for alloc in nc.m.functions[0].allocations:
            if not isinstance(alloc, mybir.MemoryLocationSet):
                continue
            name = alloc.memorylocations[0].name
            if alloc.kind == "ExternalInput":
                if name != partition_name:
                    in_names.append(name)
            elif alloc.kind == "ExternalOutput":
                out_names.append(name)
                out_avals.append(
                    jax.core.ShapedArray(
                        tuple(alloc.tensor_shape), mybir.dt.np(alloc.dtype)
                    )
                )
        self.in_names = list(in_names)
        self.out_names = out_names
        self.out_avals = out_avals
        n_params = len(in_names)
        n_outs = len(out_names)
        all_names = in_names + out_names
        if partition_name is not None:
            all_names = all_names + [partition_name]

        def _body(*args):
            operands = list(args)
            if partition_name is not None:
                operands.append(bass2jax.partition_id_tensor())
            outs = bass2jax._bass_exec_p.bind(
                *operands,
                out_avals=tuple(out_avals),
                in_names=tuple(all_names),
                out_names=tuple(out_names),
                lowering_input_output_aliases=(),
                sim_require_finite=True,
                sim_require_nnan=True,
                nc=nc,
            )
            return tuple(outs)

        devices = jax.devices()[:NCORES]
        self.mesh = mesh = Mesh(np.asarray(devices), ("core",))
        # xT is batch-sharded along axis 0; cT and bias are replicated
        # (uploaded once, not 8x); outputs are sharded.
        in_specs_by_name = {
            "xT": PartitionSpec("core"),
            "cT": PartitionSpec(),
            "bias": PartitionSpec(),
        }
        in_specs = tuple(in_specs_by_name[n] for n in in_names) + (
            PartitionSpec("core"),
        ) * n_outs
        out_specs = (PartitionSpec("core"),) * n_outs

        def _make_jit():
            return jax.jit(
                shard_map(
                    _body,
                    mesh=mesh,
                    in_specs=in_specs,
                    out_specs=out_specs,
                    check_rep=False,
                ),
                donate_argnums=tuple(range(n_params, n_params + n_outs)),
                keep_unused=True,
            )

        self._make_jit = _make_jit
        self._fn = _make_jit()

    def _zeros(self):
        return [
            np.zeros((NCORES * a.shape[0], *a.shape[1:]), a.dtype)
            for a in self.out_avals
        ]

    def fast_fn(self, example_args):
        """AOT-compiled C++ fast-dispatch variant of _fn (bass_effect
        suppressed) — much lower per-call dispatch overhead."""
        if getattr(self, "_fast", None) is None:
            from concourse import bass2jax

            self._fast = bass2jax.fast_dispatch_compile(
                lambda: self._make_jit().lower(*example_args).compile()
            )
        return self._fast

    def device_inputs(self, xT_all, cT, bias):
        """Pre-place the inputs on the devices with the expected shardings."""
        import jax
        from jax.sharding import NamedSharding, PartitionSpec

        by_name = {"xT": xT_all, "cT": cT, "bias": bias}
        spec_by_name = {
            "xT": PartitionSpec("core"),
            "cT": PartitionSpec(),
            "bias": PartitionSpec(),
        }
        out = [
            jax.device_put(
                by_name[n], NamedSharding(self.mesh, spec_by_name[n])
            )
            for n in self.in_names
        ]
        jax.block_until_ready(out)
        return out

    def run(self, xT_all, cT, bias):
        """xT_all: [NCORES*IN, BS] (core-sharded), cT: [IN, OUT], bias: [1, OUT].
        Returns y [B, OUT]."""
        out_arrs = self._fn(xT_all, cT, bias, *self._zeros())
        (y,) = [np.asarray(a) for a in out_arrs]
        return y

    def timed_call(self, dev_in, fast=True):
        """One timed call with device-resident inputs (zeros staged outside
        the timed region). Returns (seconds, out_arrs)."""
        import time

        import jax
        from jax.sharding import NamedSharding, PartitionSpec

        sh = NamedSharding(self.mesh, PartitionSpec("core"))
        zeros = [jax.device_put(z, sh) for z in self._zeros()]
        jax.block_until_ready(zeros)
        fn = self.fast_fn(tuple(dev_in) + tuple(zeros)) if fast else self._fn
        t0 = time.perf_counter()
        out_arrs = fn(*dev_in, *zeros)
        jax.block_until_ready(out_arrs)
        return time.perf_counter() - t0, out_arrs

    def run_timed(self, dev_in, iters=5, fast=True):
        """Steady-state exec timing with device-resident inputs. Returns
        (times_s, y)."""
        times = []
        out_arrs = None
        for _ in range(iters):
            dt, out_arrs = self.timed_call(dev_in, fast=fast)
            times.append(dt)
        y = np.asarray(out_arrs[0])
        return times, y


def _runtime():
    if "rt" not in _CACHE:
        _CACHE["rt"] = _Runtime()
    return _CACHE["rt"]


def _prep_inputs(x, c, bias):
    """Host-side shard/layout prep: returns (xT_all [8*IN, BS], cT [IN, OUT],
    bias [1, OUT])."""
    x = np.asarray(x, dtype=np.float32)
    c = np.asarray(c, dtype=np.float32)
    bias2 = np.ascontiguousarray(
        np.asarray(bias, dtype=np.float32).reshape(1, OUT)
    )

    sigma = (-np.arange(IN)) % IN
    # cT[nidx, o] = c[o, sigma[nidx]]  (transpose + circulant permutation)
    cT = np.ascontiguousarray(c[:, sigma].T)

    # per-core transposed shards, stacked along axis 0 for shard_map
    xT_all = np.ascontiguousarray(
        x.reshape(NCORES, BS, IN).transpose(0, 2, 1).reshape(NCORES * IN, BS)
    )
    return xT_all, cT, bias2


def kernel(x, c, bias):
    rt = _runtime()
    xT_all, cT, bias2 = _prep_inputs(x, c, bias)
    try:
        return rt.run(xT_all, cT, bias2)
    except Exception:
        # transient device errors (e.g. a wedged exec unit from an earlier
        # tenant) sometimes clear on retry
        import time as _t

        _t.sleep(2)
        return rt.run(xT_all, cT, bias2)
